# revision 1
# baseline (speedup 1.0000x reference)
"""Trainium2 Bass kernel for LocalXLAttention (chunk-summed variant).

Math: the reference einsum sums over the chunk index z, so every query
attends to the same three [w, dh] K/V matrices built from chunk sums:
  K_prev = S_k - k_chunk[C-1], K_cur = S_k, K_next = S_k - k_chunk[0]
(and identically for V), where S_k = sum_c k_chunk[c].  The computation
collapses to, per sequence position l and head h:
  attn[l,h,:]  = qp[l,h,:] @ KbigT          (KbigT: [dh, 3w])
  probs        = softmax(attn, axis=-1)
  ctx[l,h,:]   = probs[l,h,:] @ Vbig        (Vbig:  [3w, dh])
  out          = ctx.reshape(L, dm) @ Wc

Sharding: L=4096 is split 512 rows per core across 8 NeuronCores
(data-parallel over the sequence; no collectives).  Each core redundantly
computes the tiny chunk-summed K/V from the full kv input.

The attention pipeline runs fully transposed ([j, l] / [he, l] layouts) so
no on-device transposes of activations are needed; probs normalization is
deferred to the context (an extra all-ones column of Vbig accumulates the
softmax denominator for free).

Matmuls run in float32r (TF32-class PE mode, 1 cycle/row vs 4 for fp32).
"""

import sys
for _p in ('/opt/pypackages', '/opt/trn_rl_repo'):
    if _p not in sys.path:
        sys.path.insert(0, _p)

import numpy as np

import concourse.bass as bass
import concourse.bacc as bacc
import concourse.tile as tile
from concourse import mybir
from concourse.bass_utils import run_bass_kernel_spmd
from concourse.masks import make_identity

F32 = mybir.dt.float32
F32R = mybir.dt.float32r
AF = mybir.ActivationFunctionType

N_CORES = 8
L = 4096          # full sequence
LS = L // N_CORES # 512 rows per core
DM = 1024
NH = 16
DH = 64
W = 512           # chunk width
C = L // W        # 8 chunks
J3 = 3 * W        # 1536 softmax width
NJ = J3 // 128    # 12 j-chunks
DMT = DM // 128   # 8 dm-chunks


def build_nc():
    nc = bacc.Bacc(None, target_bir_lowering=False)

    qT = nc.dram_tensor("qT", [DM, LS], F32R, kind="ExternalInput")
    kvT = nc.dram_tensor("kvT", [DM, L], F32R, kind="ExternalInput")
    Wq = nc.dram_tensor("Wq", [DM, DM], F32R, kind="ExternalInput")
    Wkv = nc.dram_tensor("Wkv", [DM, 2 * DH], F32R, kind="ExternalInput")
    Wc = nc.dram_tensor("Wc", [DM, DM], F32R, kind="ExternalInput")
    out = nc.dram_tensor("out", [LS, DM], F32, kind="ExternalOutput")

    with tile.TileContext(nc) as tc:
        with tc.tile_pool(name="weights", bufs=8) as wpool, \
             tc.tile_pool(name="small", bufs=1) as spool, \
             tc.tile_pool(name="qp", bufs=8) as qpool, \
             tc.tile_pool(name="qpt", bufs=4) as qptpool, \
             tc.tile_pool(name="stream", bufs=2) as stpool, \
             tc.tile_pool(name="kvsum", bufs=8) as kvspool, \
             tc.tile_pool(name="var", bufs=4) as varpool, \
             tc.tile_pool(name="probs", bufs=4) as ppool, \
             tc.tile_pool(name="misc", bufs=2) as mpool, \
             tc.tile_pool(name="dram", bufs=1, space="DRAM") as dpool, \
             tc.tile_pool(name="psacc", bufs=4, space="PSUM") as psacc, \
             tc.tile_pool(name="psmm", bufs=2, space="PSUM") as psmm:

            # ---------- load weights / q ----------
            wq_sb = []
            for d in range(DMT):
                t = wpool.tile([128, DM], F32R, tag="wq", name=f"wq{d}")
                nc.gpsimd.dma_start(out=t, in_=Wq[128 * d:128 * (d + 1), :])
                wq_sb.append(t)
            wkv_sb = []
            for d in range(DMT):
                t = wpool.tile([128, 2 * DH], F32R, tag="wkv", name=f"wkv{d}")
                nc.sync.dma_start(out=t, in_=Wkv[128 * d:128 * (d + 1), :])
                wkv_sb.append(t)
            qt_sb = []
            for d in range(DMT):
                t = qpool.tile([128, LS], F32R, tag="qt", name=f"qt{d}")
                nc.gpsimd.dma_start(out=t, in_=qT[128 * d:128 * (d + 1), :])
                qt_sb.append(t)

            ident = spool.tile([128, 128], F32, tag="ident")
            make_identity(nc, ident)

            # ---------- kv stream: chunk-sum (tree, in place) ----------
            # kvsum_sb[d][p, y] = sum_c kvT[128d+p, 512c + y]
            kvsum_sb = []
            k7p = psacc.tile([128, W], F32, tag="acc", name="k7p")
            v7p = psacc.tile([128, W], F32, tag="acc", name="v7p")
            for d in range(DMT):
                st = stpool.tile([128, L], F32R, tag="kvstream")
                nc.sync.dma_start(out=st[:, 0:L // 2],
                                  in_=kvT[128 * d:128 * (d + 1), 0:L // 2])
                nc.scalar.dma_start(out=st[:, L // 2:L],
                                    in_=kvT[128 * d:128 * (d + 1), L // 2:L])
                # chunk-7 columns [3584:4096] are only read (never written) by
                # the in-place tree below, so project k7/v7 straight from the
                # stream tile instead of reloading those columns later.
                nc.tensor.matmul(k7p[0:DH, :], wkv_sb[d][:, 0:DH],
                                 st[:, L - W:L], start=(d == 0),
                                 stop=(d == DMT - 1))
                nc.tensor.matmul(v7p[0:DH, :], wkv_sb[d][:, DH:2 * DH],
                                 st[:, L - W:L], start=(d == 0),
                                 stop=(d == DMT - 1))
                nc.vector.tensor_add(st[:, 0:2048], st[:, 0:2048], st[:, 2048:4096])
                nc.vector.tensor_add(st[:, 0:1024], st[:, 0:1024], st[:, 1024:2048])
                ks = kvspool.tile([128, W], F32R, tag="kvsum")
                nc.vector.tensor_add(ks, st[:, 0:512], st[:, 512:1024])
                kvsum_sb.append(ks)
            k7_sb = spool.tile([DH, W], F32, tag="k7")
            v7_sb = spool.tile([DH, W], F32, tag="v7")
            nc.vector.tensor_copy(k7_sb, k7p[0:DH, :])
            nc.vector.tensor_copy(v7_sb, v7p[0:DH, :])

            # ---------- QP_T = Wq.T @ q.T  (unscaled; 1/sqrt(dh) folded into exp) ----
            # qpt_sb[t][p, 512*half + l] = QP_T[hd = 128*(2t+half) + p, l]
            qpt_sb = []
            for t4 in range(4):
                ps = psmm.tile([128, 1024], F32, tag="mm")
                for half in range(2):
                    hd = 2 * t4 + half
                    for d in range(DMT):
                        nc.tensor.matmul(
                            ps[:, 512 * half:512 * (half + 1)],
                            wq_sb[d][:, 128 * hd:128 * (hd + 1)],
                            qt_sb[d],
                            start=(d == 0), stop=(d == DMT - 1))
                sb = qptpool.tile([128, 1024], F32R, tag="qpt")
                nc.vector.tensor_copy(sb, ps)
                qpt_sb.append(sb)

            # ---------- chunk-0 / chunk-7 K,V projections ----------
            # reload kvT columns for chunks 0 and 7 (the stream tiles are
            # mutated in place by the tree sum and rotate away).
            def project_variant(rhs_tiles, tag):
                """returns psum tiles (k [64,512], v [64,512]) accumulated
                over the 8 dm chunks of rhs_tiles (each [128, 512])."""
                kp = psacc.tile([128, W], F32, tag="acc")
                vp = psacc.tile([128, W], F32, tag="acc")
                for d in range(DMT):
                    nc.tensor.matmul(kp[0:DH, :], wkv_sb[d][:, 0:DH],
                                     rhs_tiles[d], start=(d == 0),
                                     stop=(d == DMT - 1))
                    nc.tensor.matmul(vp[0:DH, :], wkv_sb[d][:, DH:2 * DH],
                                     rhs_tiles[d], start=(d == 0),
                                     stop=(d == DMT - 1))
                return kp, vp

            kv0_sb = []
            for d in range(DMT):
                t0 = varpool.tile([128, W], F32R, tag="kv07", name=f"kv0_{d}")
                nc.scalar.dma_start(out=t0, in_=kvT[128 * d:128 * (d + 1), 0:W])
                kv0_sb.append(t0)

            k0_ps, v0_ps = project_variant(kv0_sb, "c0")
            # evacuate immediately so the psum slots can rotate
            k0_sb = spool.tile([DH, W], F32, tag="k0")
            v0_sb = spool.tile([DH, W], F32, tag="v0")
            nc.vector.tensor_copy(k0_sb, k0_ps[0:DH, :])
            nc.vector.tensor_copy(v0_sb, v0_ps[0:DH, :])

            ksum_ps, vsum_ps = project_variant(kvsum_sb, "sum")
            vsum_sb = spool.tile([DH, W], F32, tag="vsum")
            nc.vector.tensor_copy(vsum_sb, vsum_ps[0:DH, :])

            # ---------- KbigT [64, 1536] = [prev | cur | next] ----------
            # duplicated into partitions 64:128 so heads whose QP_T rows sit
            # at base partition 64 get a base-matched lhsT.
            kbig = spool.tile([128, J3], F32R, tag="kbig")
            nc.vector.tensor_sub(kbig[0:DH, 0:W], ksum_ps[0:DH, :], k7_sb)
            nc.vector.tensor_copy(kbig[0:DH, W:2 * W], ksum_ps[0:DH, :])
            nc.vector.tensor_sub(kbig[0:DH, 2 * W:3 * W], ksum_ps[0:DH, :], k0_sb)
            nc.vector.tensor_copy(kbig[DH:2 * DH, :], kbig[0:DH, :])

            # ---------- Vbig [128, 12, 65(+pad)] ----------
            # chunk j rows p: j-index 128j + p of the 1536; col 64 = ones
            # (softmax denominator accumulator).
            vbig = spool.tile([128, NJ, 68], F32R, tag="vbig")
            ones_sb = spool.tile([128, 1], F32, tag="ones")
            nc.vector.memset(ones_sb, 1.0)
            for j in range(NJ):
                nc.vector.tensor_copy(vbig[:, j, DH:DH + 1], ones_sb)
            for yt in range(4):
                tps = psacc.tile([128, W], F32, tag="acc")
                tp0 = psacc.tile([128, W], F32, tag="acc")
                tp7 = psacc.tile([128, W], F32, tag="acc")
                sl = slice(128 * yt, 128 * (yt + 1))
                nc.tensor.transpose(tps[:, 0:DH], vsum_sb[:, sl], ident[0:DH, 0:DH])
                nc.tensor.transpose(tp0[:, 0:DH], v0_sb[:, sl], ident[0:DH, 0:DH])
                nc.tensor.transpose(tp7[:, 0:DH], v7_sb[:, sl], ident[0:DH, 0:DH])
                # DVE may read only one PSUM operand: evacuate cur first,
                # then subtract the other transposes against the SBUF copy.
                nc.vector.tensor_copy(vbig[:, 4 + yt, 0:DH], tps[:, 0:DH])
                nc.vector.tensor_sub(vbig[:, 0 + yt, 0:DH], vbig[:, 4 + yt, 0:DH], tp7[:, 0:DH])
                nc.vector.tensor_sub(vbig[:, 8 + yt, 0:DH], vbig[:, 4 + yt, 0:DH], tp0[:, 0:DH])

            # ---------- attention (transposed): QK -> exp -> PV ----------
            # denominator rows go through a DRAM scratch because engine APs
            # need 32-aligned base partitions (can't write row h directly).
            dscratch = dpool.tile([NH, W], F32, name="dscratch")
            ctxu_sb = []  # 8 pair tiles [128, 512]: rows 0:64 head 2t, 64:128 head 2t+1
            for t in range(8):
                ctxu_sb.append(qpool.tile([128, W], F32R, tag="qt", name=f"ctxu{t}"))

            for t in range(8):  # head pairs (2t, 2t+1)
                qpt = qpt_sb[t // 2]
                csl = slice(512 * (t % 2), 512 * (t % 2) + W)
                rhsA = qpt[0:DH, csl]
                rhsB = qpt[DH:2 * DH, csl]
                ctxA = psacc.tile([128, W], F32, tag="acc", name=f"ctxA{t}")
                ctxB = psacc.tile([128, W], F32, tag="acc", name=f"ctxB{t}")
                for j in range(NJ):
                    qk = psmm.tile([128, 1024], F32, tag="mm", name=f"qk{t}_{j}")
                    # row-packed pair: even head on PE rows 0:64, odd head on
                    # rows 64:128 (tile_position auto-derived from base
                    # partitions) -> both matmuls run concurrently.
                    nc.tensor.matmul(qk[:, 0:W],
                                     kbig[0:DH, 128 * j:128 * (j + 1)],
                                     rhsA, start=True, stop=True)
                    nc.tensor.matmul(qk[:, W:2 * W],
                                     kbig[DH:2 * DH, 128 * j:128 * (j + 1)],
                                     rhsB, start=True, stop=True)
                    pr = ppool.tile([128, 1024], F32R, tag="probs", name=f"pr{t}_{j}")
                    nc.scalar.activation(pr, qk, AF.Exp, scale=0.125)
                    nc.tensor.matmul(ctxA[0:DH + 1, :], vbig[:, j, 0:DH + 1],
                                     pr[:, 0:W],
                                     start=(j == 0), stop=(j == NJ - 1))
                    nc.tensor.matmul(ctxB[0:DH + 1, :], vbig[:, j, 0:DH + 1],
                                     pr[:, W:2 * W],
                                     start=(j == 0), stop=(j == NJ - 1))
                for h, ctx_ps in ((2 * t, ctxA), (2 * t + 1, ctxB)):
                    dtmp = mpool.tile([1, W], F32, tag="dtmp", name=f"dtmp{h}", bufs=1)
                    nc.vector.tensor_copy(dtmp, ctx_ps[DH:DH + 1, :])
                    nc.sync.dma_start(out=dscratch[h:h + 1, :], in_=dtmp)
                    nc.vector.tensor_copy(
                        ctxu_sb[h // 2][64 * (h % 2):64 * (h % 2) + DH, :],
                        ctx_ps[0:DH, :])
                if t % 2 == 1:
                    # normalize the 2 pairs (4 heads) whose denominators are
                    # complete; earlier batches overlap later pairs' compute.
                    b0 = 4 * (t // 2)
                    dn = mpool.tile([4, W], F32, tag="dn", name=f"dn{t}", bufs=1)
                    nc.scalar.dma_start(out=dn, in_=dscratch[b0:b0 + 4, :])
                    rc = mpool.tile([4, W], F32, tag="rc", name=f"rc{t}", bufs=1)
                    nc.vector.reciprocal(rc, dn)
                    rsc = dpool.tile([4, W], F32, name=f"rsc{t}")
                    nc.scalar.dma_start(out=rsc, in_=rc)
                    for pt in (t - 1, t):
                        bc = mpool.tile([128, W], F32, tag="bcast", name=f"bc{pt}")
                        src = bass.AP(tensor=rsc.tensor,
                                      offset=rsc.offset + (2 * pt - b0) * W,
                                      ap=[[W, 2], [0, DH], [1, W]])
                        nc.scalar.dma_start(out=bc, in_=src)
                        nc.vector.tensor_mul(ctxu_sb[pt], ctxu_sb[pt], bc)

            # ---------- out = ctx @ Wc ----------
            wc_sb = []
            for d in range(DMT):
                t = wpool.tile([128, DM], F32R, tag="wc", name=f"wc{d}")
                nc.gpsimd.dma_start(out=t, in_=Wc[128 * d:128 * (d + 1), :])
                wc_sb.append(t)

            for lt in range(LS // 128):
                ps = psmm.tile([128, 1024], F32, tag="mm")
                for half in range(2):
                    for he in range(DMT):
                        nc.tensor.matmul(
                            ps[:, 512 * half:512 * (half + 1)],
                            ctxu_sb[he][:, 128 * lt:128 * (lt + 1)],
                            wc_sb[he][:, 512 * half:512 * (half + 1)],
                            start=(he == 0), stop=(he == DMT - 1))
                ob = mpool.tile([128, DM], F32, tag="outsb", bufs=1)
                nc.vector.tensor_copy(ob, ps)
                nc.sync.dma_start(out=out[128 * lt:128 * (lt + 1), :], in_=ob)

    nc.compile()
    return nc


_NC = None


def _get_nc():
    global _NC
    if _NC is None:
        _NC = build_nc()
    return _NC


def kernel(q, kv, Wq, Wkv, Wc, w):
    assert int(w) == W
    q = np.asarray(q, dtype=np.float32)
    kv = np.asarray(kv, dtype=np.float32)
    B = q.shape[0]
    assert B == 1 and q.shape[1] == L and q.shape[2] == DM

    qT_full = np.ascontiguousarray(q[0].T)    # [DM, L]
    kvT = np.ascontiguousarray(kv[0].T)       # [DM, L]
    Wq = np.ascontiguousarray(Wq, dtype=np.float32)
    Wkv = np.ascontiguousarray(Wkv, dtype=np.float32)
    Wc = np.ascontiguousarray(Wc, dtype=np.float32)

    in_maps = []
    for i in range(N_CORES):
        in_maps.append({
            "qT": np.ascontiguousarray(qT_full[:, LS * i:LS * (i + 1)]),
            "kvT": kvT,
            "Wq": Wq,
            "Wkv": Wkv,
            "Wc": Wc,
        })

    nc = _get_nc()
    res = run_bass_kernel_spmd(nc, in_maps, list(range(N_CORES)))
    out = np.concatenate([res.results[i]["out"] for i in range(N_CORES)], axis=0)
    return out.reshape(1, L, DM).astype(np.float32)



# revision 3
# speedup vs baseline: 1.1822x; 1.1822x over previous
"""Trainium2 Bass kernel for LocalXLAttention (chunk-summed variant).

Math: the reference einsum sums over the chunk index z, so every query
attends to the same three [w, dh] K/V matrices built from chunk sums:
  K_prev = S_k - k_chunk[C-1], K_cur = S_k, K_next = S_k - k_chunk[0]
(and identically for V), where S_k = sum_c k_chunk[c].  The computation
collapses to, per sequence position l and head h:
  attn[l,h,:]  = qp[l,h,:] @ KbigT          (KbigT: [dh, 3w])
  probs        = softmax(attn, axis=-1)
  ctx[l,h,:]   = probs[l,h,:] @ Vbig        (Vbig:  [3w, dh])
  out          = ctx.reshape(L, dm) @ Wc

Sharding: L=4096 is split 512 rows per core across 8 NeuronCores
(data-parallel over the sequence; no collectives).  Each core redundantly
computes the tiny chunk-summed K/V from the full kv input.

All inputs are cast to bf16 on the host (halves HBM traffic; matmuls run
at 1 cycle/row).  The attention pipeline runs fully transposed
([j, l] / [he, l] layouts) so no on-device transposes of activations are
needed; probs normalization is deferred to the context (an extra all-ones
column of Vbig accumulates the softmax denominator for free).

PSUM budget (8 banks):
  tag "qk"  2 x [128,1024] f32  (4 banks) - QK ping/pong, QP projection
  tag "ctx" 2 x [128, 512] f32  (2 banks) - per-pair ctx accumulators
  tag "op"  2 x [128, 512] f32  (2 banks) - out-proj partials, kv-sum
                                            projection, vbig transposes

Pipeline: per head-pair p, 12 j-chunks of QK (TensorE, row-tiled pairs)
-> exp (ScalarE, the overall bottleneck at ~12.6M exps) -> PV (TensorE,
psum-accumulated).  The out-projection of pair p-1 is deferred into pair
p's j-loop so the ScalarE queue never starves; partial outputs accumulate
in SBUF via VectorE.
"""

import sys
for _p in ('/opt/pypackages', '/opt/trn_rl_repo'):
    if _p not in sys.path:
        sys.path.insert(0, _p)

import numpy as np
import ml_dtypes

import concourse.bass as bass
import concourse.bacc as bacc
import concourse.tile as tile
from concourse import mybir
from concourse.bass_utils import run_bass_kernel_spmd
from concourse.masks import make_identity

F32 = mybir.dt.float32
BF16 = mybir.dt.bfloat16
AF = mybir.ActivationFunctionType

N_CORES = 8
L = 4096          # full sequence
LS = L // N_CORES # 512 rows per core
DM = 1024
NH = 16
DH = 64
W = 512           # chunk width
C = L // W        # 8 chunks
J3 = 3 * W        # 1536 softmax width
NJ = J3 // 128    # 12 j-chunks
DMT = DM // 128   # 8 dm-chunks


def build_nc():
    nc = bacc.Bacc(None, target_bir_lowering=False)

    qT = nc.dram_tensor("qT", [DM, LS], BF16, kind="ExternalInput")
    kvT = nc.dram_tensor("kvT", [DM, L], BF16, kind="ExternalInput")
    Wq = nc.dram_tensor("Wq", [DM, DM], BF16, kind="ExternalInput")
    Wkv = nc.dram_tensor("Wkv", [DM, 2 * DH], BF16, kind="ExternalInput")
    Wc = nc.dram_tensor("Wc", [DM, DM], BF16, kind="ExternalInput")
    out = nc.dram_tensor("out", [LS, DM], F32, kind="ExternalOutput")

    with tile.TileContext(nc) as tc:
        with tc.tile_pool(name="weights", bufs=8) as wpool, \
             tc.tile_pool(name="qt", bufs=8) as qpool, \
             tc.tile_pool(name="stream", bufs=8) as stpool, \
             tc.tile_pool(name="ksum", bufs=8) as kspool, \
             tc.tile_pool(name="qpt", bufs=4) as qptpool, \
             tc.tile_pool(name="small", bufs=1) as spool, \
             tc.tile_pool(name="probs", bufs=3) as ppool, \
             tc.tile_pool(name="ctxu", bufs=2) as cupool, \
             tc.tile_pool(name="outacc", bufs=4) as opool, \
             tc.tile_pool(name="misc", bufs=4) as mpool, \
             tc.tile_pool(name="ps", bufs=1, space="PSUM") as pspool:

            # ---------- phase 0: input DMAs ----------
            # sync (HWDGE): q + Wq first (gate QP), then half the kv stream.
            # scalar (HWDGE ring 2): other half of kv stream (ScalarE is idle
            # until the first exp).  gpsimd (SWDGE): Wkv + Wc (needed late).
            qt_sb = []
            for d in range(DMT):
                t = qpool.tile([128, LS], BF16, tag="qt", name=f"qt{d}")
                nc.sync.dma_start(out=t, in_=qT[128 * d:128 * (d + 1), :])
                qt_sb.append(t)
            wq_sb = []
            for d in range(DMT):
                t = wpool.tile([128, DM], BF16, tag="wq", name=f"wq{d}")
                nc.sync.dma_start(out=t, in_=Wq[128 * d:128 * (d + 1), :])
                wq_sb.append(t)
            wkv_sb = []
            for d in range(DMT):
                t = wpool.tile([128, 2 * DH], BF16, tag="wkv", name=f"wkv{d}")
                nc.gpsimd.dma_start(out=t, in_=Wkv[128 * d:128 * (d + 1), :])
                wkv_sb.append(t)
            st_sb = []
            for d in range(DMT):
                t = stpool.tile([128, L], BF16, tag="st", name=f"st{d}")
                eng = nc.sync if d % 2 == 0 else nc.scalar
                eng.dma_start(out=t, in_=kvT[128 * d:128 * (d + 1), :])
                st_sb.append(t)
            wc_sb = []
            for d in range(DMT):
                t = wpool.tile([128, DM], BF16, tag="wc", name=f"wc{d}")
                nc.gpsimd.dma_start(out=t, in_=Wc[128 * d:128 * (d + 1), :])
                wc_sb.append(t)

            ident = spool.tile([128, 128], BF16, tag="ident")
            make_identity(nc, ident)

            # preload the exp table so the first real exp isn't delayed ~2.7us
            dummy = mpool.tile([1, 8], F32, tag="dummy")
            nc.scalar.activation(dummy, ident[0:1, 0:8], AF.Exp, scale=1.0)

            # ---------- phase 1+2 (interleaved on PE): QP and kv projections --
            # kv7p/kv0p: [128, 512] psum, k rows 0:64, v rows 64:128 (the v
            # matmuls are col-tiled to base partition 64).  Accumulated over
            # the 8 dm-chunks of the kv stream.
            kv7p = pspool.tile([128, W], F32, tag="ctx", bufs=2, name="kv7p")
            kv0p = pspool.tile([128, W], F32, tag="ctx", bufs=2, name="kv0p")

            def proj_chunk(d):
                st = st_sb[d]
                first, last = (d == 0), (d == DMT - 1)
                # chunk 7 (cols L-W:L) is never written by the in-place tree
                nc.tensor.matmul(kv7p[0:DH, :], wkv_sb[d][:, 0:DH],
                                 st[:, L - W:L], start=first, stop=last)
                nc.tensor.matmul(kv7p[DH:128, :], wkv_sb[d][:, DH:2 * DH],
                                 st[:, L - W:L], start=first, stop=last)
                # chunk 0 must be projected before the tree overwrites it
                nc.tensor.matmul(kv0p[0:DH, :], wkv_sb[d][:, 0:DH],
                                 st[:, 0:W], start=first, stop=last)
                nc.tensor.matmul(kv0p[DH:128, :], wkv_sb[d][:, DH:2 * DH],
                                 st[:, 0:W], start=first, stop=last)

            def tree_chunk(d):
                # in-place bf16 chunk-sum tree: ks[d][p, y] = sum_c st[p, 512c+y]
                st = st_sb[d]
                nc.vector.tensor_add(st[:, 0:2048], st[:, 0:2048], st[:, 2048:4096])
                nc.vector.tensor_add(st[:, 0:1024], st[:, 0:1024], st[:, 1024:2048])
                ks = kspool.tile([128, W], BF16, tag="ks", name=f"ks{d}")
                nc.vector.tensor_add(ks, st[:, 0:512], st[:, 512:1024])
                return ks

            qpt_sb = []

            def qp_quad(t4):
                ps = pspool.tile([128, 1024], F32, tag="qk", bufs=2, name=f"qp{t4}")
                for half in range(2):
                    hd = 2 * t4 + half
                    for d in range(DMT):
                        nc.tensor.matmul(
                            ps[:, 512 * half:512 * (half + 1)],
                            wq_sb[d][:, 128 * hd:128 * (hd + 1)],
                            qt_sb[d],
                            start=(d == 0), stop=(d == DMT - 1))
                sb = qptpool.tile([128, 1024], BF16, tag="qpt", name=f"qpt{t4}")
                nc.vector.tensor_copy(sb, ps)
                qpt_sb.append(sb)

            ks_sb = [None] * DMT
            # PE program order: early kv chunks, then QP (waits only on q/Wq),
            # then the remaining kv chunks as their DMAs land.
            for d in range(3):
                proj_chunk(d)
                ks_sb[d] = tree_chunk(d)
            qp_quad(0)
            qp_quad(1)
            for d in range(3, DMT):
                proj_chunk(d)
                ks_sb[d] = tree_chunk(d)
            qp_quad(2)
            qp_quad(3)

            # chunk-sum projection (k rows 0:64, v rows 64:128)
            ksump = pspool.tile([128, W], F32, tag="op", bufs=2, name="ksump")
            for d in range(DMT):
                nc.tensor.matmul(ksump[0:DH, :], wkv_sb[d][:, 0:DH],
                                 ks_sb[d], start=(d == 0), stop=(d == DMT - 1))
                nc.tensor.matmul(ksump[DH:128, :], wkv_sb[d][:, DH:2 * DH],
                                 ks_sb[d], start=(d == 0), stop=(d == DMT - 1))

            # evacuate chunk 0/7 projections
            k7_sb = spool.tile([DH, W], BF16, tag="k7")
            v7_sb = spool.tile([DH, W], BF16, tag="v7")
            k0_sb = spool.tile([DH, W], BF16, tag="k0")
            v0_sb = spool.tile([DH, W], BF16, tag="v0")
            nc.vector.tensor_copy(k7_sb, kv7p[0:DH, :])
            nc.vector.tensor_copy(v7_sb, kv7p[DH:128, :])
            nc.vector.tensor_copy(k0_sb, kv0p[0:DH, :])
            nc.vector.tensor_copy(v0_sb, kv0p[DH:128, :])

            # ---------- KbigT [128, 1536] = [prev | cur | next] ----------
            # rows 0:64 hold the real thing; rows 64:128 are a copy so the
            # odd head of each pair gets a base-64-matched lhsT (row tiling).
            kbig = spool.tile([128, J3], BF16, tag="kbig")
            nc.vector.tensor_sub(kbig[0:DH, 0:W], ksump[0:DH, :], k7_sb)
            nc.vector.tensor_copy(kbig[0:DH, W:2 * W], ksump[0:DH, :])
            nc.vector.tensor_sub(kbig[0:DH, 2 * W:3 * W], ksump[0:DH, :], k0_sb)
            nc.vector.tensor_copy(kbig[DH:2 * DH, :], kbig[0:DH, :])
            vsum_sb = spool.tile([DH, W], BF16, tag="vsum")
            nc.vector.tensor_copy(vsum_sb, ksump[DH:128, :])

            # ---------- Vbig [128, 12, 68] ----------
            # j-chunk j rows p hold Vbig row 128j+p; col 64 = ones (softmax
            # denominator accumulator); cols 65:68 padding.
            vbig = spool.tile([128, NJ, 68], BF16, tag="vbig")
            nc.vector.memset(vbig[:, :, DH:DH + 1], 1.0)
            for yt in range(4):
                sl = slice(128 * yt, 128 * (yt + 1))
                tps = pspool.tile([128, DH], BF16, tag="op", bufs=2, name=f"tps{yt}")
                nc.tensor.transpose(tps, vsum_sb[:, sl], ident[0:DH, 0:DH])
                nc.vector.tensor_copy(vbig[:, 4 + yt, 0:DH], tps)
                tp7 = pspool.tile([128, DH], BF16, tag="op", bufs=2, name=f"tp7{yt}")
                nc.tensor.transpose(tp7, v7_sb[:, sl], ident[0:DH, 0:DH])
                nc.vector.tensor_sub(vbig[:, 0 + yt, 0:DH], vbig[:, 4 + yt, 0:DH], tp7)
                tp0 = pspool.tile([128, DH], BF16, tag="op", bufs=2, name=f"tp0{yt}")
                nc.tensor.transpose(tp0, v0_sb[:, sl], ident[0:DH, 0:DH])
                nc.vector.tensor_sub(vbig[:, 8 + yt, 0:DH], vbig[:, 4 + yt, 0:DH], tp0)

            # ---------- main attention loop ----------
            outacc = []
            for lt in range(4):
                t = opool.tile([128, DM], F32, tag="outacc", name=f"outacc{lt}")
                outacc.append(t)
            ctxu_sb = [None] * 8

            def emit_outproj(p):
                cu = ctxu_sb[p]
                for lt in range(4):
                    for half in range(2):
                        op = pspool.tile([128, W], F32, tag="op", bufs=2,
                                         name=f"op{p}_{lt}_{half}")
                        nc.tensor.matmul(
                            op, cu[:, 128 * lt:128 * (lt + 1)],
                            wc_sb[p][:, 512 * half:512 * (half + 1)],
                            start=True, stop=True)
                        dst = outacc[lt][:, 512 * half:512 * (half + 1)]
                        if p == 0:
                            nc.vector.tensor_copy(dst, op)
                        else:
                            nc.vector.tensor_add(dst, dst, op)
                    if p == 7:
                        nc.sync.dma_start(out=out[128 * lt:128 * (lt + 1), :],
                                          in_=outacc[lt])

            for p in range(8):  # head pairs (2p, 2p+1)
                qpt = qpt_sb[p // 2]
                csl = slice(512 * (p % 2), 512 * (p % 2) + W)
                ctxA = pspool.tile([128, W], F32, tag="ctx", bufs=2, name=f"ctxA{p}")
                ctxB = pspool.tile([128, W], F32, tag="ctx", bufs=2, name=f"ctxB{p}")
                for j in range(NJ):
                    qk = pspool.tile([128, 1024], F32, tag="qk", bufs=2, name=f"qk{p}_{j}")
                    # row-packed pair: even head on PE rows 0:64, odd head on
                    # rows 64:128 (tile_position auto-derived from base
                    # partitions) -> both matmuls run concurrently.
                    nc.tensor.matmul(qk[:, 0:W],
                                     kbig[0:DH, 128 * j:128 * (j + 1)],
                                     qpt[0:DH, csl], start=True, stop=True)
                    nc.tensor.matmul(qk[:, W:2 * W],
                                     kbig[DH:2 * DH, 128 * j:128 * (j + 1)],
                                     qpt[DH:2 * DH, csl], start=True, stop=True)
                    pr = ppool.tile([128, 1024], BF16, tag="probs",
                                    name=f"pr{p}_{j}")
                    nc.scalar.activation(pr, qk, AF.Exp, scale=0.125)
                    nc.tensor.matmul(ctxA[0:DH + 1, :], vbig[:, j, 0:DH + 1],
                                     pr[:, 0:W],
                                     start=(j == 0), stop=(j == NJ - 1))
                    nc.tensor.matmul(ctxB[0:DH + 1, :], vbig[:, j, 0:DH + 1],
                                     pr[:, W:2 * W],
                                     start=(j == 0), stop=(j == NJ - 1))
                    if j == 3 and p >= 1:
                        # deferred out-projection of the previous pair: keeps
                        # PE fed without starving the ScalarE exp queue at the
                        # pair boundary.
                        emit_outproj(p - 1)
                # normalize: row 64 of ctx psum = softmax denominator
                cu = cupool.tile([128, W], BF16, tag="ctxu", name=f"ctxu{p}")
                ctxu_sb[p] = cu
                for h_idx, ctp in ((0, ctxA), (1, ctxB)):
                    dtmp = mpool.tile([1, W], F32, tag="dtmp", name=f"dt{p}_{h_idx}")
                    nc.vector.tensor_copy(dtmp, ctp[DH:DH + 1, :])
                    rc = mpool.tile([1, W], F32, tag="rc", name=f"rc{p}_{h_idx}")
                    nc.vector.reciprocal(rc, dtmp)
                    bc = mpool.tile([DH, W], F32, tag="bc", name=f"bc{p}_{h_idx}")
                    nc.gpsimd.partition_broadcast(bc, rc)
                    nc.vector.tensor_mul(cu[DH * h_idx:DH * (h_idx + 1), :],
                                         ctp[0:DH, :], bc)
            emit_outproj(7)

    nc.compile()
    return nc


_NC = None


def _get_nc():
    global _NC
    if _NC is None:
        _NC = build_nc()
    return _NC


def make_in_maps(q, kv, Wq, Wkv, Wc):
    bf = ml_dtypes.bfloat16
    qT_full = np.ascontiguousarray(np.asarray(q, np.float32)[0].T.astype(bf))
    kvT = np.ascontiguousarray(np.asarray(kv, np.float32)[0].T.astype(bf))
    Wqb = np.ascontiguousarray(np.asarray(Wq, np.float32).astype(bf))
    Wkvb = np.ascontiguousarray(np.asarray(Wkv, np.float32).astype(bf))
    Wcb = np.ascontiguousarray(np.asarray(Wc, np.float32).astype(bf))
    in_maps = []
    for i in range(N_CORES):
        in_maps.append({
            "qT": np.ascontiguousarray(qT_full[:, LS * i:LS * (i + 1)]),
            "kvT": kvT,
            "Wq": Wqb,
            "Wkv": Wkvb,
            "Wc": Wcb,
        })
    return in_maps


def kernel(q, kv, Wq, Wkv, Wc, w):
    assert int(w) == W
    q = np.asarray(q, dtype=np.float32)
    B = q.shape[0]
    assert B == 1 and q.shape[1] == L and q.shape[2] == DM

    in_maps = make_in_maps(q, kv, Wq, Wkv, Wc)
    nc = _get_nc()
    res = run_bass_kernel_spmd(nc, in_maps, list(range(N_CORES)))
    out = np.concatenate([res.results[i]["out"] for i in range(N_CORES)], axis=0)
    return out.reshape(1, L, DM).astype(np.float32)


# revision 17
# speedup vs baseline: 1.3096x; 1.1078x over previous
"""Trainium2 Bass kernel for LocalXLAttention (chunk-summed variant).

Math: the reference einsum sums over the chunk index z, so every query
attends to the same three [w, dh] K/V matrices built from chunk sums:
  K_prev = S_k - k_chunk[C-1], K_cur = S_k, K_next = S_k - k_chunk[0]
(and identically for V), where S_k = sum_c k_chunk[c].  The computation
collapses to, per sequence position l and head h:
  attn[l,h,:]  = qp[l,h,:] @ KbigT          (KbigT: [dh, 3w])
  probs        = softmax(attn, axis=-1)
  ctx[l,h,:]   = probs[l,h,:] @ Vbig        (Vbig:  [3w, dh])
  out          = ctx.reshape(L, dm) @ Wc
with the scores factored per block:
  exp(u - a) = exp(u)*exp(-a),  u = qp.S^T,  a = qp.c7^T,  b = qp.c0^T
so the a/b exponentials of the first head pairs run while the kv chunk-sum
S is still streaming in (the exp unit is the overall bottleneck at ~12.6M
exps/core; it must start early and never stall).

Sharding: L=4096 is split 512 rows per core across 8 NeuronCores
(data-parallel over the sequence; no collectives).  Each core redundantly
computes the tiny chunk-summed K/V from the full kv input.  All inputs are
cast to bf16 on the host; chunks 0 and 7 of kv are fetched first as
dedicated tiles (they gate the a/b scores), the middle 6 chunks stream
behind them.

Layouts are fully transposed ([j, l] / [he, l]) so no on-device
activation transposes are needed; softmax normalization is deferred to
the context via an all-ones column of Vbig (row 64 of the ctx psum
accumulates the denominator for free).

PSUM budget (8 banks):
  tag "qk"  3 x [128,1024] f32  (6 banks) - QK/exp ping-pong-pang, QP
                                            projection, out-proj partials
  tag "ctx" 2 x [128, 512] f32  (2 banks) - per-pair ctx accumulators,
                                            kv projections, vbig transposes

Pipeline per head pair: QK (TensorE, row-tiled pairs) -> exp (ScalarE)
-> PV (TensorE, psum-accumulated).  Pair ctx is raw-evicted to SBUF at
pair end (fast psum release), normalized lazily (approx reciprocal +
gpsimd broadcast), and the out-projection is spread across the next
pair's j-loop; partial outputs accumulate in SBUF (bf16) via VectorE.
"""

import sys
for _p in ('/opt/pypackages', '/opt/trn_rl_repo'):
    if _p not in sys.path:
        sys.path.insert(0, _p)

import numpy as np
import ml_dtypes

import concourse.bass as bass
import concourse.bacc as bacc
import concourse.tile as tile
from concourse import mybir
from concourse.bass_utils import run_bass_kernel_spmd
from concourse.masks import make_identity

F32 = mybir.dt.float32
BF16 = mybir.dt.bfloat16
AF = mybir.ActivationFunctionType

N_CORES = 8
L = 4096          # full sequence
LS = L // N_CORES # 512 rows per core
DM = 1024
NH = 16
DH = 64
W = 512           # chunk width
C = L // W        # 8 chunks
J3 = 3 * W        # 1536 softmax width
NJ = J3 // 128    # 12 j-chunks
DMT = DM // 128   # 8 dm-chunks
MID = L - 2 * W   # 3072 middle columns (chunks 1..6)
N_EAB = 2         # head pairs that run the decomposed (early) schedule


def build_nc():
    nc = bacc.Bacc(None, target_bir_lowering=False)

    qT = nc.dram_tensor("qT", [DM, LS], BF16, kind="ExternalInput")
    kvT = nc.dram_tensor("kvT", [DM, L], BF16, kind="ExternalInput")
    Wq = nc.dram_tensor("Wq", [DM, DM], BF16, kind="ExternalInput")
    Wkv = nc.dram_tensor("Wkv", [DM, 2 * DH], BF16, kind="ExternalInput")
    Wc = nc.dram_tensor("Wc", [DM, DM], BF16, kind="ExternalInput")
    # bf16 output (cast during the SWDGE DMA) halves the output-write
    # tail; the host upcasts to fp32.
    out = nc.dram_tensor("out", [LS, DM], BF16, kind="ExternalOutput")

    with tile.TileContext(nc) as tc:
        with tc.tile_pool(name="weights", bufs=8) as wpool, \
             tc.tile_pool(name="qt", bufs=8) as qpool, \
             tc.tile_pool(name="stream", bufs=8) as stpool, \
             tc.tile_pool(name="kvc", bufs=16) as kvcpool, \
             tc.tile_pool(name="ksum", bufs=8) as kspool, \
             tc.tile_pool(name="qpt", bufs=4) as qptpool, \
             tc.tile_pool(name="small", bufs=1) as spool, \
             tc.tile_pool(name="probs", bufs=4) as ppool, \
             tc.tile_pool(name="eab", bufs=16) as eabpool, \
             tc.tile_pool(name="craw", bufs=2) as crpool, \
             tc.tile_pool(name="ctxu", bufs=2) as cupool, \
             tc.tile_pool(name="outacc", bufs=4) as opool, \
             tc.tile_pool(name="misc", bufs=2) as mpool, \
             tc.tile_pool(name="ps", bufs=1, space="PSUM") as pspool:

            # ---------- phase 0: input DMAs (both HWDGE rings) ----------
            # ring FIFO order is the priority order:
            #   sync:   qt, Wq-lo, kvc0, Wq-hi, stm0, stm2, Wc
            #   scalar: wkv, kvc7, stm1, stm3..stm7
            qt_sb = []
            for d in range(DMT):
                t = qpool.tile([128, LS], BF16, tag="qt", name=f"qt{d}")
                nc.sync.dma_start(out=t, in_=qT[128 * d:128 * (d + 1), :])
                qt_sb.append(t)
            wkv_sb = []
            for d in range(DMT):
                t = wpool.tile([128, 2 * DH], BF16, tag="wkv", name=f"wkv{d}")
                nc.scalar.dma_start(out=t, in_=Wkv[128 * d:128 * (d + 1), :])
                wkv_sb.append(t)
            wq_sb = []
            for d in range(DMT):
                t = wpool.tile([128, DM], BF16, tag="wq", name=f"wq{d}")
                nc.sync.dma_start(out=t[:, 0:512],
                                  in_=Wq[128 * d:128 * (d + 1), 0:512])
                wq_sb.append(t)
            kvc7_sb = []
            for d in range(DMT):
                t = kvcpool.tile([128, W], BF16, tag="kvc", name=f"kvc7_{d}")
                nc.scalar.dma_start(out=t, in_=kvT[128 * d:128 * (d + 1), L - W:L])
                kvc7_sb.append(t)
            kvc0_sb = []
            for d in range(DMT):
                t = kvcpool.tile([128, W], BF16, tag="kvc", name=f"kvc0_{d}")
                nc.sync.dma_start(out=t, in_=kvT[128 * d:128 * (d + 1), 0:W])
                kvc0_sb.append(t)
            for d in range(DMT):
                nc.sync.dma_start(out=wq_sb[d][:, 512:1024],
                                  in_=Wq[128 * d:128 * (d + 1), 512:1024])
            stm_sb = []
            for d in range(DMT):
                t = stpool.tile([128, MID], BF16, tag="st", name=f"stm{d}")
                eng = nc.sync if d in (0, 2) else nc.scalar
                eng.dma_start(out=t, in_=kvT[128 * d:128 * (d + 1), W:L - W])
                stm_sb.append(t)
            wc_sb = []
            for d in range(DMT):
                t = wpool.tile([128, DM], BF16, tag="wc", name=f"wc{d}")
                nc.sync.dma_start(out=t, in_=Wc[128 * d:128 * (d + 1), :])
                wc_sb.append(t)

            ident = spool.tile([128, 128], BF16, tag="ident")
            make_identity(nc, ident)

            # preload the exp table so the first real exp isn't delayed ~2.7us
            dummy = mpool.tile([1, 8], F32, tag="dummy")
            nc.scalar.activation(dummy, ident[0:1, 0:8], AF.Exp, scale=1.0)

            # ---------- chunk 7 / chunk 0 projections (early) ----------
            # [128, 512] psum: k rows 0:64, v rows 64:128 (v matmuls are
            # col-tiled to base partition 64), accumulated over dm-chunks.
            kv7p = pspool.tile([128, W], F32, tag="ctx", bufs=2, name="kv7p")
            kv0p = pspool.tile([128, W], F32, tag="ctx", bufs=2, name="kv0p")
            for src, dst in ((kvc7_sb, kv7p), (kvc0_sb, kv0p)):
                for d in range(DMT):
                    nc.tensor.matmul(dst[0:DH, :], wkv_sb[d][:, 0:DH],
                                     src[d], start=(d == 0), stop=(d == DMT - 1))
                    nc.tensor.matmul(dst[DH:128, :], wkv_sb[d][:, DH:2 * DH],
                                     src[d], start=(d == 0), stop=(d == DMT - 1))

            # a/b-score lhsT tiles (rows duplicated for the row-tiled pair)
            # + v7/v0 for the Vbig transposes
            k7b = spool.tile([128, W], BF16, tag="k7b")
            k0b = spool.tile([128, W], BF16, tag="k0b")
            v7_sb = spool.tile([DH, W], BF16, tag="v7")
            v0_sb = spool.tile([DH, W], BF16, tag="v0")
            nc.vector.tensor_copy(k7b[0:DH, :], kv7p[0:DH, :])
            nc.vector.tensor_copy(k7b[DH:128, :], k7b[0:DH, :])
            nc.vector.tensor_copy(v7_sb, kv7p[DH:128, :])
            nc.vector.tensor_copy(k0b[0:DH, :], kv0p[0:DH, :])
            nc.vector.tensor_copy(k0b[DH:128, :], k0b[0:DH, :])
            nc.vector.tensor_copy(v0_sb, kv0p[DH:128, :])

            # ---------- QP projection (by head quads) ----------
            qpt_sb = []

            def qp_quad(t4):
                ps = pspool.tile([128, 1024], F32, tag="qk", bufs=3, name=f"qp{t4}")
                for half in range(2):
                    hd = 2 * t4 + half
                    for d in range(DMT):
                        nc.tensor.matmul(
                            ps[:, 512 * half:512 * (half + 1)],
                            wq_sb[d][:, 128 * hd:128 * (hd + 1)],
                            qt_sb[d],
                            start=(d == 0), stop=(d == DMT - 1))
                sb = qptpool.tile([128, 1024], BF16, tag="qpt", name=f"qpt{t4}")
                nc.vector.tensor_copy(sb, ps)
                qpt_sb.append(sb)

            def qk_mm_pair(lhsT, qpt, csl, name):
                qk = pspool.tile([128, 1024], F32, tag="qk", bufs=3, name=name)
                nc.tensor.matmul(qk[:, 0:W], lhsT[0:DH, :],
                                 qpt[0:DH, csl], start=True, stop=True)
                nc.tensor.matmul(qk[:, W:2 * W], lhsT[DH:128, :],
                                 qpt[DH:2 * DH, csl], start=True, stop=True)
                return qk

            # ---------- alpha phase: a/b exponentials of pairs 0..N_EAB-1 --
            # Ea = exp(-0.125*a), Eb = exp(-0.125*b); multiplied by
            # Eu = exp(0.125*u) later, once the chunk-sum S lands.
            ea_t = [[None] * 4 for _ in range(N_EAB)]
            eb_t = [[None] * 4 for _ in range(N_EAB)]

            def alpha_block(p, blk):
                qpt = qpt_sb[p // 2]
                csl = slice(512 * (p % 2), 512 * (p % 2) + W)
                lhsT, store = ((k7b, ea_t) if blk == 0 else (k0b, eb_t))
                for jj in range(4):
                    qk = qk_mm_pair(lhsT[:, 128 * jj:128 * (jj + 1)], qpt, csl,
                                    f"abqk{p}_{blk}_{jj}")
                    e = eabpool.tile([128, 1024], BF16, tag="eab",
                                     name=f"e{p}_{blk}_{jj}")
                    nc.scalar.activation(e, qk, AF.Exp, scale=-0.125)
                    store[p][jj] = e

            qp_quad(0)
            alpha_block(0, 0)
            alpha_block(0, 1)
            alpha_block(1, 0)
            alpha_block(1, 1)

            # ---------- chunk-sum tree (middle chunks + c0 + c7) ----------
            ks_sb = []
            for d in range(DMT):
                stm = stm_sb[d]
                nc.vector.tensor_add(stm[:, 0:1536], stm[:, 0:1536],
                                     stm[:, 1536:3072])
                nc.vector.tensor_add(stm[:, 0:512], stm[:, 0:512],
                                     stm[:, 512:1024])
                ks = kspool.tile([128, W], BF16, tag="ks", name=f"ks{d}")
                nc.vector.tensor_add(ks, stm[:, 0:512], stm[:, 1024:1536])
                nc.vector.tensor_add(ks, ks, kvc0_sb[d])
                nc.vector.tensor_add(ks, ks, kvc7_sb[d])
                ks_sb.append(ks)

            ksump = pspool.tile([128, W], F32, tag="ctx", bufs=2, name="ksump")
            for d in range(DMT):
                nc.tensor.matmul(ksump[0:DH, :], wkv_sb[d][:, 0:DH],
                                 ks_sb[d], start=(d == 0), stop=(d == DMT - 1))
                nc.tensor.matmul(ksump[DH:128, :], wkv_sb[d][:, DH:2 * DH],
                                 ks_sb[d], start=(d == 0), stop=(d == DMT - 1))

            # ---------- KbigT [128, 1536] = [prev | cur | next] ----------
            kbig = spool.tile([128, J3], BF16, tag="kbig")
            nc.vector.tensor_sub(kbig[0:DH, 0:W], ksump[0:DH, :], k7b[0:DH, :])
            nc.vector.tensor_copy(kbig[0:DH, W:2 * W], ksump[0:DH, :])
            nc.vector.tensor_sub(kbig[0:DH, 2 * W:3 * W], ksump[0:DH, :],
                                 k0b[0:DH, :])
            nc.vector.tensor_copy(kbig[DH:2 * DH, :], kbig[0:DH, :])
            vsum_sb = spool.tile([DH, W], BF16, tag="vsum")
            nc.vector.tensor_copy(vsum_sb, ksump[DH:128, :])

            qp_quad(1)

            # ---------- Vbig [128, 12, 68] ----------
            # j-chunk j rows p hold Vbig row 128j+p; col 64 = ones (softmax
            # denominator accumulator); cols 65:68 padding.
            vbig = spool.tile([128, NJ, 68], BF16, tag="vbig")
            nc.vector.memset(vbig[:, :, DH:DH + 1], 1.0)
            for yt in range(4):
                sl = slice(128 * yt, 128 * (yt + 1))
                tps = pspool.tile([128, DH], BF16, tag="ctx", bufs=2,
                                  name=f"tps{yt}")
                nc.tensor.transpose(tps, vsum_sb[:, sl], ident[0:DH, 0:DH])
                nc.vector.tensor_copy(vbig[:, 4 + yt, 0:DH], tps)
                tp7 = pspool.tile([128, DH], BF16, tag="ctx", bufs=2,
                                  name=f"tp7{yt}")
                nc.tensor.transpose(tp7, v7_sb[:, sl], ident[0:DH, 0:DH])
                nc.vector.tensor_sub(vbig[:, 0 + yt, 0:DH],
                                     vbig[:, 4 + yt, 0:DH], tp7)
                tp0 = pspool.tile([128, DH], BF16, tag="ctx", bufs=2,
                                  name=f"tp0{yt}")
                nc.tensor.transpose(tp0, v0_sb[:, sl], ident[0:DH, 0:DH])
                nc.vector.tensor_sub(vbig[:, 8 + yt, 0:DH],
                                     vbig[:, 4 + yt, 0:DH], tp0)

            # ---------- main attention machinery ----------
            outacc = []
            for lt in range(4):
                t = opool.tile([128, DM], BF16, tag="outacc", name=f"outacc{lt}")
                outacc.append(t)
            ctxu_sb = [None] * 8

            def emit_outproj(p, lt):
                # out-proj partials share the "qk" psum slots; one merged
                # [128,1024] bf16 DVE accumulate per l-tile.  Spread one
                # l-tile at a time through the next pair's j-loop so the DVE
                # evictions never monopolize the qk slots.
                cu = ctxu_sb[p]
                op = pspool.tile([128, 1024], F32, tag="qk", bufs=3,
                                 name=f"op{p}_{lt}")
                for half in range(2):
                    nc.tensor.matmul(
                        op[:, 512 * half:512 * (half + 1)],
                        cu[:, 128 * lt:128 * (lt + 1)],
                        wc_sb[p][:, 512 * half:512 * (half + 1)],
                        start=True, stop=True)
                if p == 0:
                    nc.vector.tensor_copy(outacc[lt], op)
                else:
                    nc.vector.tensor_add(outacc[lt], outacc[lt], op)
                if p == 7:
                    nc.gpsimd.dma_start(out=out[128 * lt:128 * (lt + 1), :],
                                        in_=outacc[lt])

            def make_ctx(p):
                ctxA = pspool.tile([128, W], F32, tag="ctx", bufs=2,
                                   name=f"ctxA{p}")
                ctxB = pspool.tile([128, W], F32, tag="ctx", bufs=2,
                                   name=f"ctxB{p}")
                return ctxA, ctxB

            def pv_mm(ctxA, ctxB, j, pr, start, stop):
                nc.tensor.matmul(ctxA[0:DH + 1, :], vbig[:, j, 0:DH + 1],
                                 pr[:, 0:W], start=start, stop=stop)
                nc.tensor.matmul(ctxB[0:DH + 1, :], vbig[:, j, 0:DH + 1],
                                 pr[:, W:2 * W], start=start, stop=stop)

            def finish_pair(p, ctxA, ctxB):
                # raw-evict ctx psum (fast slot release), then normalize
                # lazily from SBUF: row 64 is the softmax denominator.
                cu = cupool.tile([128, W], BF16, tag="ctxu", name=f"ctxu{p}")
                ctxu_sb[p] = cu
                for h_idx, ctp in ((0, ctxA), (1, ctxB)):
                    cr = crpool.tile([DH, W], F32, tag="craw",
                                     name=f"cr{p}_{h_idx}")
                    nc.vector.tensor_copy(cr, ctp[0:DH, :])
                    dtmp = mpool.tile([1, W], F32, tag="dtmp",
                                      name=f"dt{p}_{h_idx}")
                    nc.vector.tensor_copy(dtmp, ctp[DH:DH + 1, :])
                    rc = mpool.tile([1, W], F32, tag="rc", name=f"rc{p}_{h_idx}")
                    nc.vector.reciprocal_approx_fast(rc, dtmp)
                    bc = mpool.tile([DH, W], F32, tag="bc", name=f"bc{p}_{h_idx}")
                    nc.gpsimd.partition_broadcast(bc, rc)
                    nc.vector.tensor_mul(cu[DH * h_idx:DH * (h_idx + 1), :],
                                         cr, bc)

            def beta_pair(p, outproj_of=None):
                # u-exponentials + recombination + PV for a decomposed pair
                qpt = qpt_sb[p // 2]
                csl = slice(512 * (p % 2), 512 * (p % 2) + W)
                ctxA, ctxB = make_ctx(p)
                for jj in range(4):
                    qk = qk_mm_pair(kbig[:, W + 128 * jj:W + 128 * (jj + 1)],
                                    qpt, csl, f"uqk{p}_{jj}")
                    eu = ppool.tile([128, 1024], BF16, tag="probs",
                                    name=f"eu{p}_{jj}")
                    nc.scalar.activation(eu, qk, AF.Exp, scale=0.125)
                    nc.vector.tensor_mul(ea_t[p][jj], ea_t[p][jj], eu)
                    nc.vector.tensor_mul(eb_t[p][jj], eb_t[p][jj], eu)
                    pv_mm(ctxA, ctxB, 4 + jj, eu, start=(jj == 0), stop=False)
                    pv_mm(ctxA, ctxB, 0 + jj, ea_t[p][jj], start=False,
                          stop=False)
                    pv_mm(ctxA, ctxB, 8 + jj, eb_t[p][jj], start=False,
                          stop=(jj == 3))
                    if outproj_of is not None and jj > 0:
                        emit_outproj(outproj_of, jj - 1)
                if outproj_of is not None:
                    emit_outproj(outproj_of, 3)
                finish_pair(p, ctxA, ctxB)

            def std_pair(p, extra=None):
                qpt = qpt_sb[p // 2]
                csl = slice(512 * (p % 2), 512 * (p % 2) + W)
                ctxA, ctxB = make_ctx(p)
                for j in range(NJ):
                    qk = qk_mm_pair(kbig[:, 128 * j:128 * (j + 1)], qpt, csl,
                                    f"qk{p}_{j}")
                    pr = ppool.tile([128, 1024], BF16, tag="probs",
                                    name=f"pr{p}_{j}")
                    nc.scalar.activation(pr, qk, AF.Exp, scale=0.125)
                    pv_mm(ctxA, ctxB, j, pr, start=(j == 0), stop=(j == NJ - 1))
                    if j in (3, 5, 7, 9):
                        emit_outproj(p - 1, (j - 3) // 2)
                    if j == 6 and extra is not None:
                        extra()  # QP quads ride the ACT-saturated j-loop
                finish_pair(p, ctxA, ctxB)

            beta_pair(0)
            beta_pair(1, outproj_of=0)
            std_pair(2, extra=lambda: qp_quad(2))
            std_pair(3, extra=lambda: qp_quad(3))
            for p in range(4, 8):
                std_pair(p)
            for lt in range(4):
                emit_outproj(7, lt)

    nc.compile()
    return nc


_NC = None


def _get_nc():
    global _NC
    if _NC is None:
        _NC = build_nc()
    return _NC


def make_in_maps(q, kv, Wq, Wkv, Wc):
    bf = ml_dtypes.bfloat16
    qT_full = np.ascontiguousarray(np.asarray(q, np.float32)[0].T.astype(bf))
    kvT = np.ascontiguousarray(np.asarray(kv, np.float32)[0].T.astype(bf))
    Wqb = np.ascontiguousarray(np.asarray(Wq, np.float32).astype(bf))
    Wkvb = np.ascontiguousarray(np.asarray(Wkv, np.float32).astype(bf))
    Wcb = np.ascontiguousarray(np.asarray(Wc, np.float32).astype(bf))
    in_maps = []
    for i in range(N_CORES):
        in_maps.append({
            "qT": np.ascontiguousarray(qT_full[:, LS * i:LS * (i + 1)]),
            "kvT": kvT,
            "Wq": Wqb,
            "Wkv": Wkvb,
            "Wc": Wcb,
        })
    return in_maps


def kernel(q, kv, Wq, Wkv, Wc, w):
    assert int(w) == W
    q = np.asarray(q, dtype=np.float32)
    B = q.shape[0]
    assert B == 1 and q.shape[1] == L and q.shape[2] == DM

    in_maps = make_in_maps(q, kv, Wq, Wkv, Wc)
    nc = _get_nc()
    res = run_bass_kernel_spmd(nc, in_maps, list(range(N_CORES)))
    out = np.concatenate([np.asarray(res.results[i]["out"], dtype=np.float32)
                          for i in range(N_CORES)], axis=0)
    return out.reshape(1, L, DM)


# revision 19
# speedup vs baseline: 1.4654x; 1.1190x over previous
"""Trainium2 Bass kernel for LocalXLAttention (chunk-summed variant).

Math: the reference einsum sums over the chunk index z, so every query
attends to the same three [w, dh] K/V matrices built from chunk sums:
  K_prev = S_k - k_chunk[C-1], K_cur = S_k, K_next = S_k - k_chunk[0]
(and identically for V), where S_k = sum_c k_chunk[c].  The computation
collapses to, per sequence position l and head h:
  attn[l,h,:]  = qp[l,h,:] @ KbigT          (KbigT: [dh, 3w])
  probs        = softmax(attn, axis=-1)
  ctx[l,h,:]   = probs[l,h,:] @ Vbig        (Vbig:  [3w, dh])
  out          = ctx.reshape(L, dm) @ Wc
with the scores factored per block:
  exp(u - a) = exp(u)*exp(-a),  u = qp.S^T,  a = qp.c7^T,  b = qp.c0^T
so the a/b exponentials of the first head pairs run while the kv chunk-sum
S is still streaming in (the exp unit is the overall bottleneck at ~12.6M
exps/core; it must start early and never stall).

Sharding: L=4096 is split 512 rows per core across 8 NeuronCores
(data-parallel over the sequence; no collectives).  Each core redundantly
computes the tiny chunk-summed K/V from the full kv input.  All inputs are
cast to bf16 on the host; chunks 0 and 7 of kv are fetched first as
dedicated tiles (they gate the a/b scores), the middle 6 chunks stream
behind them.

Layouts are fully transposed ([j, l] / [he, l]) so no on-device
activation transposes are needed; softmax normalization is deferred to
the context via an all-ones column of Vbig (row 64 of the ctx psum
accumulates the denominator for free).

PSUM budget (8 banks):
  tag "qk"  3 x [128,1024] f32  (6 banks) - QK/exp ping-pong-pang, QP
                                            projection, out-proj partials
  tag "ctx" 2 x [128, 512] f32  (2 banks) - per-pair ctx accumulators,
                                            kv projections, vbig transposes

Pipeline per head pair: QK (TensorE, row-tiled pairs) -> exp (ScalarE)
-> PV (TensorE, psum-accumulated).  Pair ctx is raw-evicted to SBUF at
pair end (fast psum release), normalized lazily (approx reciprocal +
gpsimd broadcast), and the out-projection is spread across the next
pair's j-loop; partial outputs accumulate in SBUF (bf16) via VectorE.
"""

import sys
for _p in ('/opt/pypackages', '/opt/trn_rl_repo'):
    if _p not in sys.path:
        sys.path.insert(0, _p)

import numpy as np
import ml_dtypes

import concourse.bass as bass
import concourse.bacc as bacc
import concourse.tile as tile
from concourse import mybir
from concourse.bass_utils import run_bass_kernel_spmd
from concourse.masks import make_identity

F32 = mybir.dt.float32
BF16 = mybir.dt.bfloat16
AF = mybir.ActivationFunctionType

N_CORES = 8
L = 4096          # full sequence
LS = L // N_CORES # 512 rows per core
DM = 1024
NH = 16
DH = 64
W = 512           # chunk width
C = L // W        # 8 chunks
J3 = 3 * W        # 1536 softmax width
NJ = J3 // 128    # 12 j-chunks
DMT = DM // 128   # 8 dm-chunks
MID = L - 2 * W   # 3072 middle columns (chunks 1..6)
N_EAB = 2         # head pairs that run the decomposed (early) schedule


def build_nc():
    nc = bacc.Bacc(None, target_bir_lowering=False)

    qT = nc.dram_tensor("qT", [DM, LS], BF16, kind="ExternalInput")
    kvT = nc.dram_tensor("kvT", [DM, L], BF16, kind="ExternalInput")
    Wq = nc.dram_tensor("Wq", [DM, DM], BF16, kind="ExternalInput")
    Wkv = nc.dram_tensor("Wkv", [DM, 2 * DH], BF16, kind="ExternalInput")
    Wc = nc.dram_tensor("Wc", [DM, DM], BF16, kind="ExternalInput")
    # bf16 output (cast during the SWDGE DMA) halves the output-write
    # tail; the host upcasts to fp32.
    out = nc.dram_tensor("out", [LS, DM], BF16, kind="ExternalOutput")

    with tile.TileContext(nc) as tc:
        with tc.tile_pool(name="weights", bufs=1) as wpool, \
             tc.tile_pool(name="qt", bufs=1) as qpool, \
             tc.tile_pool(name="stream", bufs=8) as stpool, \
             tc.tile_pool(name="kvc", bufs=2) as kvcpool, \
             tc.tile_pool(name="ksum", bufs=8) as kspool, \
             tc.tile_pool(name="qpt", bufs=4) as qptpool, \
             tc.tile_pool(name="small", bufs=1) as spool, \
             tc.tile_pool(name="probs", bufs=4) as ppool, \
             tc.tile_pool(name="eab", bufs=16) as eabpool, \
             tc.tile_pool(name="craw", bufs=2) as crpool, \
             tc.tile_pool(name="ctxu", bufs=2) as cupool, \
             tc.tile_pool(name="outacc", bufs=4) as opool, \
             tc.tile_pool(name="misc", bufs=2) as mpool, \
             tc.tile_pool(name="ps", bufs=1, space="PSUM") as pspool:

            # ---------- phase 0: input DMAs (both HWDGE rings) ----------
            # ring FIFO order is the priority order:
            #   sync:   qt, Wq-lo, kvc0, Wq-hi, stm0, stm2, Wc
            #   scalar: wkv, kvc7, stm1, stm3..stm7
            def ap3d(dram, cols, coff):
                # [128, 8, cols] view of a [DM, X] dram tensor: partition p,
                # dm-chunk d, column c  ->  dram[128*d + p, coff + c]
                X = dram.shape[-1]
                return bass.AP(tensor=dram, offset=coff,
                               ap=[[X, 128], [128 * X, DMT], [1, cols]])

            qt3 = qpool.tile([128, DMT, LS], BF16, tag="qt")
            nc.sync.dma_start(out=qt3, in_=ap3d(qT, LS, 0))
            wkv3 = wpool.tile([128, DMT, 2 * DH], BF16, tag="wkv")
            nc.scalar.dma_start(out=wkv3, in_=ap3d(Wkv, 2 * DH, 0))
            wq3 = wpool.tile([128, DMT, DM], BF16, tag="wq")
            nc.sync.dma_start(out=wq3[:, :, 0:512], in_=ap3d(Wq, 512, 0))
            kvc73 = kvcpool.tile([128, DMT, W], BF16, tag="kvc")
            nc.scalar.dma_start(out=kvc73, in_=ap3d(kvT, W, L - W))
            kvc03 = kvcpool.tile([128, DMT, W], BF16, tag="kvc")
            nc.sync.dma_start(out=kvc03, in_=ap3d(kvT, W, 0))
            nc.sync.dma_start(out=wq3[:, :, 512:1024], in_=ap3d(Wq, 512, 512))
            stm_sb = []
            for d in range(DMT):
                t = stpool.tile([128, MID], BF16, tag="st", name=f"stm{d}")
                eng = nc.sync if d in (0, 2) else nc.scalar
                eng.dma_start(out=t, in_=kvT[128 * d:128 * (d + 1), W:L - W])
                stm_sb.append(t)
            wc3 = wpool.tile([128, DMT, DM], BF16, tag="wc")
            nc.sync.dma_start(out=wc3, in_=ap3d(Wc, DM, 0))
            qt_sb = [qt3[:, d, :] for d in range(DMT)]
            wkv_sb = [wkv3[:, d, :] for d in range(DMT)]
            wq_sb = [wq3[:, d, :] for d in range(DMT)]
            kvc7_sb = [kvc73[:, d, :] for d in range(DMT)]
            kvc0_sb = [kvc03[:, d, :] for d in range(DMT)]
            wc_sb = [wc3[:, d, :] for d in range(DMT)]

            ident = spool.tile([128, 128], BF16, tag="ident")
            make_identity(nc, ident)

            # preload the exp table so the first real exp isn't delayed ~2.7us
            dummy = mpool.tile([1, 8], F32, tag="dummy")
            nc.scalar.activation(dummy, ident[0:1, 0:8], AF.Exp, scale=1.0)

            # ---------- chunk 7 / chunk 0 projections (early) ----------
            # [128, 512] psum: k rows 0:64, v rows 64:128 (v matmuls are
            # col-tiled to base partition 64), accumulated over dm-chunks.
            kv7p = pspool.tile([128, W], F32, tag="ctx", bufs=2, name="kv7p")
            kv0p = pspool.tile([128, W], F32, tag="ctx", bufs=2, name="kv0p")
            for src, dst in ((kvc7_sb, kv7p), (kvc0_sb, kv0p)):
                for d in range(DMT):
                    nc.tensor.matmul(dst[0:DH, :], wkv_sb[d][:, 0:DH],
                                     src[d], start=(d == 0), stop=(d == DMT - 1))
                    nc.tensor.matmul(dst[DH:128, :], wkv_sb[d][:, DH:2 * DH],
                                     src[d], start=(d == 0), stop=(d == DMT - 1))

            # a/b-score lhsT tiles (rows duplicated for the row-tiled pair)
            # + v7/v0 for the Vbig transposes
            k7b = spool.tile([128, W], BF16, tag="k7b")
            k0b = spool.tile([128, W], BF16, tag="k0b")
            v7_sb = spool.tile([DH, W], BF16, tag="v7")
            v0_sb = spool.tile([DH, W], BF16, tag="v0")
            nc.vector.tensor_copy(k7b[0:DH, :], kv7p[0:DH, :])
            nc.vector.tensor_copy(k7b[DH:128, :], k7b[0:DH, :])
            nc.vector.tensor_copy(v7_sb, kv7p[DH:128, :])
            nc.vector.tensor_copy(k0b[0:DH, :], kv0p[0:DH, :])
            nc.vector.tensor_copy(k0b[DH:128, :], k0b[0:DH, :])
            nc.vector.tensor_copy(v0_sb, kv0p[DH:128, :])

            # ---------- QP projection (by head quads) ----------
            qpt_sb = []

            def qp_quad(t4):
                ps = pspool.tile([128, 1024], F32, tag="qk", bufs=3, name=f"qp{t4}")
                for half in range(2):
                    hd = 2 * t4 + half
                    for d in range(DMT):
                        nc.tensor.matmul(
                            ps[:, 512 * half:512 * (half + 1)],
                            wq_sb[d][:, 128 * hd:128 * (hd + 1)],
                            qt_sb[d],
                            start=(d == 0), stop=(d == DMT - 1))
                sb = qptpool.tile([128, 1024], BF16, tag="qpt", name=f"qpt{t4}")
                nc.vector.tensor_copy(sb, ps)
                qpt_sb.append(sb)

            def qk_mm_pair(lhsT, qpt, csl, name):
                qk = pspool.tile([128, 1024], F32, tag="qk", bufs=3, name=name)
                nc.tensor.matmul(qk[:, 0:W], lhsT[0:DH, :],
                                 qpt[0:DH, csl], start=True, stop=True)
                nc.tensor.matmul(qk[:, W:2 * W], lhsT[DH:128, :],
                                 qpt[DH:2 * DH, csl], start=True, stop=True)
                return qk

            # ---------- alpha phase: a/b exponentials of pairs 0..N_EAB-1 --
            # Ea = exp(-0.125*a), Eb = exp(-0.125*b); multiplied by
            # Eu = exp(0.125*u) later, once the chunk-sum S lands.
            ea_t = [[None] * 4 for _ in range(N_EAB)]
            eb_t = [[None] * 4 for _ in range(N_EAB)]

            def alpha_block(p, blk):
                qpt = qpt_sb[p // 2]
                csl = slice(512 * (p % 2), 512 * (p % 2) + W)
                lhsT, store = ((k7b, ea_t) if blk == 0 else (k0b, eb_t))
                for jj in range(4):
                    qk = qk_mm_pair(lhsT[:, 128 * jj:128 * (jj + 1)], qpt, csl,
                                    f"abqk{p}_{blk}_{jj}")
                    e = eabpool.tile([128, 1024], BF16, tag="eab",
                                     name=f"e{p}_{blk}_{jj}")
                    nc.scalar.activation(e, qk, AF.Exp, scale=-0.125)
                    store[p][jj] = e

            qp_quad(0)
            alpha_block(0, 0)
            alpha_block(0, 1)
            alpha_block(1, 0)
            alpha_block(1, 1)

            # ---------- chunk-sum tree (middle chunks + c0 + c7) ----------
            ks_sb = []
            for d in range(DMT):
                stm = stm_sb[d]
                nc.vector.tensor_add(stm[:, 0:1536], stm[:, 0:1536],
                                     stm[:, 1536:3072])
                nc.vector.tensor_add(stm[:, 0:512], stm[:, 0:512],
                                     stm[:, 512:1024])
                ks = kspool.tile([128, W], BF16, tag="ks", name=f"ks{d}")
                nc.vector.tensor_add(ks, stm[:, 0:512], stm[:, 1024:1536])
                nc.vector.tensor_add(ks, ks, kvc0_sb[d])
                nc.vector.tensor_add(ks, ks, kvc7_sb[d])
                ks_sb.append(ks)

            ksump = pspool.tile([128, W], F32, tag="ctx", bufs=2, name="ksump")
            for d in range(DMT):
                nc.tensor.matmul(ksump[0:DH, :], wkv_sb[d][:, 0:DH],
                                 ks_sb[d], start=(d == 0), stop=(d == DMT - 1))
                nc.tensor.matmul(ksump[DH:128, :], wkv_sb[d][:, DH:2 * DH],
                                 ks_sb[d], start=(d == 0), stop=(d == DMT - 1))

            # ---------- KbigT [128, 1536] = [prev | cur | next] ----------
            kbig = spool.tile([128, J3], BF16, tag="kbig")
            nc.vector.tensor_sub(kbig[0:DH, 0:W], ksump[0:DH, :], k7b[0:DH, :])
            nc.vector.tensor_copy(kbig[0:DH, W:2 * W], ksump[0:DH, :])
            nc.vector.tensor_sub(kbig[0:DH, 2 * W:3 * W], ksump[0:DH, :],
                                 k0b[0:DH, :])
            nc.vector.tensor_copy(kbig[DH:2 * DH, :], kbig[0:DH, :])
            vsum_sb = spool.tile([DH, W], BF16, tag="vsum")
            nc.vector.tensor_copy(vsum_sb, ksump[DH:128, :])

            qp_quad(1)

            # ---------- Vbig [128, 12, 68] ----------
            # j-chunk j rows p hold Vbig row 128j+p; col 64 = ones (softmax
            # denominator accumulator); cols 65:68 padding.
            vbig = spool.tile([128, NJ, 68], BF16, tag="vbig")
            nc.vector.memset(vbig[:, :, DH:DH + 1], 1.0)
            for yt in range(4):
                sl = slice(128 * yt, 128 * (yt + 1))
                tps = pspool.tile([128, DH], BF16, tag="ctx", bufs=2,
                                  name=f"tps{yt}")
                nc.tensor.transpose(tps, vsum_sb[:, sl], ident[0:DH, 0:DH])
                nc.vector.tensor_copy(vbig[:, 4 + yt, 0:DH], tps)
                tp7 = pspool.tile([128, DH], BF16, tag="ctx", bufs=2,
                                  name=f"tp7{yt}")
                nc.tensor.transpose(tp7, v7_sb[:, sl], ident[0:DH, 0:DH])
                nc.vector.tensor_sub(vbig[:, 0 + yt, 0:DH],
                                     vbig[:, 4 + yt, 0:DH], tp7)
                tp0 = pspool.tile([128, DH], BF16, tag="ctx", bufs=2,
                                  name=f"tp0{yt}")
                nc.tensor.transpose(tp0, v0_sb[:, sl], ident[0:DH, 0:DH])
                nc.vector.tensor_sub(vbig[:, 8 + yt, 0:DH],
                                     vbig[:, 4 + yt, 0:DH], tp0)

            # ---------- main attention machinery ----------
            outacc = []
            for lt in range(4):
                t = opool.tile([128, DM], BF16, tag="outacc", name=f"outacc{lt}")
                outacc.append(t)
            ctxu_sb = [None] * 8

            pending = []

            def pop_outproj():
                if pending:
                    emit_outproj(*pending.pop(0))

            def emit_outproj(p, lt):
                # out-proj partials share the "qk" psum slots; one merged
                # [128,1024] bf16 DVE accumulate per l-tile.  Spread one
                # l-tile at a time through the next pair's j-loop so the DVE
                # evictions never monopolize the qk slots.
                cu = ctxu_sb[p]
                op = pspool.tile([128, 1024], F32, tag="qk", bufs=3,
                                 name=f"op{p}_{lt}")
                for half in range(2):
                    nc.tensor.matmul(
                        op[:, 512 * half:512 * (half + 1)],
                        cu[:, 128 * lt:128 * (lt + 1)],
                        wc_sb[p][:, 512 * half:512 * (half + 1)],
                        start=True, stop=True)
                if p == 0:
                    nc.vector.tensor_copy(outacc[lt], op)
                else:
                    nc.vector.tensor_add(outacc[lt], outacc[lt], op)
                if p == 7:
                    nc.gpsimd.dma_start(out=out[128 * lt:128 * (lt + 1), :],
                                        in_=outacc[lt])

            def make_ctx(p):
                ctxA = pspool.tile([128, W], F32, tag="ctx", bufs=2,
                                   name=f"ctxA{p}")
                ctxB = pspool.tile([128, W], F32, tag="ctx", bufs=2,
                                   name=f"ctxB{p}")
                return ctxA, ctxB

            def pv_mm(ctxA, ctxB, j, pr, start, stop):
                nc.tensor.matmul(ctxA[0:DH + 1, :], vbig[:, j, 0:DH + 1],
                                 pr[:, 0:W], start=start, stop=stop)
                nc.tensor.matmul(ctxB[0:DH + 1, :], vbig[:, j, 0:DH + 1],
                                 pr[:, W:2 * W], start=start, stop=stop)

            def finish_pair(p, ctxA, ctxB):
                # raw-evict ctx psum (fast slot release), then normalize
                # lazily from SBUF: row 64 is the softmax denominator.
                cu = cupool.tile([128, W], BF16, tag="ctxu", name=f"ctxu{p}")
                ctxu_sb[p] = cu
                pending.extend((p, lt) for lt in range(4))
                for h_idx, ctp in ((0, ctxA), (1, ctxB)):
                    cr = crpool.tile([DH, W], BF16, tag="craw",
                                     name=f"cr{p}_{h_idx}")
                    nc.vector.tensor_copy(cr, ctp[0:DH, :])
                    dtmp = mpool.tile([1, W], F32, tag="dtmp",
                                      name=f"dt{p}_{h_idx}")
                    nc.vector.tensor_copy(dtmp, ctp[DH:DH + 1, :])
                    rc = mpool.tile([1, W], F32, tag="rc", name=f"rc{p}_{h_idx}")
                    nc.vector.reciprocal_approx_fast(rc, dtmp)
                    bc = mpool.tile([DH, W], F32, tag="bc", name=f"bc{p}_{h_idx}")
                    nc.gpsimd.partition_broadcast(bc, rc)
                    nc.vector.tensor_mul(cu[DH * h_idx:DH * (h_idx + 1), :],
                                         cr, bc)

            def beta_pair(p, outproj_of=None):
                # u-exponentials + recombination + PV for a decomposed pair
                qpt = qpt_sb[p // 2]
                csl = slice(512 * (p % 2), 512 * (p % 2) + W)
                ctxA, ctxB = make_ctx(p)
                for jj in range(4):
                    qk = qk_mm_pair(kbig[:, W + 128 * jj:W + 128 * (jj + 1)],
                                    qpt, csl, f"uqk{p}_{jj}")
                    eu = ppool.tile([128, 1024], BF16, tag="probs",
                                    name=f"eu{p}_{jj}")
                    nc.scalar.activation(eu, qk, AF.Exp, scale=0.125)
                    nc.vector.tensor_mul(ea_t[p][jj], ea_t[p][jj], eu)
                    nc.vector.tensor_mul(eb_t[p][jj], eb_t[p][jj], eu)
                    pv_mm(ctxA, ctxB, 4 + jj, eu, start=(jj == 0), stop=False)
                    pv_mm(ctxA, ctxB, 0 + jj, ea_t[p][jj], start=False,
                          stop=False)
                    pv_mm(ctxA, ctxB, 8 + jj, eb_t[p][jj], start=False,
                          stop=(jj == 3))
                    if outproj_of is not None and jj >= 2:
                        pop_outproj()
                finish_pair(p, ctxA, ctxB)

            def std_pair(p, extra=None):
                qpt = qpt_sb[p // 2]
                csl = slice(512 * (p % 2), 512 * (p % 2) + W)
                ctxA, ctxB = make_ctx(p)
                for j in range(NJ):
                    qk = qk_mm_pair(kbig[:, 128 * j:128 * (j + 1)], qpt, csl,
                                    f"qk{p}_{j}")
                    pr = ppool.tile([128, 1024], BF16, tag="probs",
                                    name=f"pr{p}_{j}")
                    nc.scalar.activation(pr, qk, AF.Exp, scale=0.125)
                    pv_mm(ctxA, ctxB, j, pr, start=(j == 0), stop=(j == NJ - 1))
                    if j in (3, 4, 5, 6):
                        pop_outproj()
                    if j == 8 and extra is not None:
                        extra()  # QP quads ride the ACT-saturated j-loop
                finish_pair(p, ctxA, ctxB)

            beta_pair(0)
            beta_pair(1, outproj_of=0)
            std_pair(2, extra=lambda: qp_quad(2))
            std_pair(3, extra=lambda: qp_quad(3))
            for p in range(4, 8):
                std_pair(p)
            while pending:
                pop_outproj()

    nc.compile()
    return nc


_NC = None


def _get_nc():
    global _NC
    if _NC is None:
        _NC = build_nc()
    return _NC


def make_in_maps(q, kv, Wq, Wkv, Wc):
    bf = ml_dtypes.bfloat16
    qT_full = np.ascontiguousarray(np.asarray(q, np.float32)[0].T.astype(bf))
    kvT = np.ascontiguousarray(np.asarray(kv, np.float32)[0].T.astype(bf))
    Wqb = np.ascontiguousarray(np.asarray(Wq, np.float32).astype(bf))
    Wkvb = np.ascontiguousarray(np.asarray(Wkv, np.float32).astype(bf))
    Wcb = np.ascontiguousarray(np.asarray(Wc, np.float32).astype(bf))
    in_maps = []
    for i in range(N_CORES):
        in_maps.append({
            "qT": np.ascontiguousarray(qT_full[:, LS * i:LS * (i + 1)]),
            "kvT": kvT,
            "Wq": Wqb,
            "Wkv": Wkvb,
            "Wc": Wcb,
        })
    return in_maps


def kernel(q, kv, Wq, Wkv, Wc, w):
    assert int(w) == W
    q = np.asarray(q, dtype=np.float32)
    B = q.shape[0]
    assert B == 1 and q.shape[1] == L and q.shape[2] == DM

    in_maps = make_in_maps(q, kv, Wq, Wkv, Wc)
    nc = _get_nc()
    res = run_bass_kernel_spmd(nc, in_maps, list(range(N_CORES)))
    out = np.concatenate([np.asarray(res.results[i]["out"], dtype=np.float32)
                          for i in range(N_CORES)], axis=0)
    return out.reshape(1, L, DM)
